# revision 36
# baseline (speedup 1.0000x reference)
"""Trainium2 Bass kernel: batched multi-head self-attention (B=16, N=1024, D=768, H=12).

Strategy
--------
Data-parallel over the batch: 16 batches / 8 NeuronCores = 2 batches per core.
Each core runs an identical (SPMD) Bass program over its shard.

Per-core math, all matmuls in bf16 with fp32 PSUM accumulation:
  * Host pre-transposes x to xT [D, T] (T = 2048 local tokens) and casts
    x / qkv_w / proj_w to bf16.  Every on-device matmul operand is then in
    its natural layout -- no on-device transposes:
      - Q^T,K^T [c, tok] = matmul(lhsT=qkv_w[:, c-tile], rhs=xT)
      - V [tok, c]       = matmul(lhsT=xT[:, tok-tile], rhs=qkv_w_v)
      - S^T [k, q]       = matmul(lhsT=K^T_h [hd, k-tile], rhs=Q^T_h [hd, q])
        (scores computed TRANSPOSED; no max-subtraction needed: |score|<~6)
      - exp on ScalarE straight out of PSUM, cast to bf16 into SBUF
      - out^T [hd, q]    = matmul(lhsT=[V_h | ones(64)], rhs=expT [k, q]);
        psum rows 64-127 = softmax denominator, replicated for free.
      - normalize via tensor_copy -> reciprocal_approx_fast -> tensor_mul
      - y [tok, e]       = matmul(lhsT=out^T [d, tok-tile], rhs=proj_w) + bias
  * Head pairs share the PE array for S^T: heads 2i/2i+1 live at partition
    bases 0/64 of Q^T/K^T, so their matmuls land on row-tiles (0,0)/(64,0)
    of the 64x128 PE configuration and execute concurrently.
  * The attention stage is ScalarE(exp)-bound, so the emission order
    software-pipelines: within a unit S^T and A@V matmuls interleave, and
    across phases batch-1 QKV / batch-0 proj units are woven between
    attention units to keep the PE busy (and HAM-warm) during exp stalls.

kernel() takes full unsharded inputs, shards on host, runs all 8 cores via
run_bass_kernel_spmd, and re-assembles the full output.
"""

import numpy as np
import ml_dtypes

import concourse.bass as bass
import concourse.mybir as mybir
import concourse.tile as tile
from concourse import bacc
from concourse.bass_utils import run_bass_kernel_spmd

BF16 = mybir.dt.bfloat16
F32 = mybir.dt.float32

N_CORES = 8
B, SEQ, D = 16, 1024, 768
H, HD = 12, 64
BPC = B // N_CORES            # batches per core
T = BPC * SEQ                 # tokens per core
P = 128
KT = D // P                   # 6 contraction sub-tiles of 128
NQ = 512                      # moving free-dim per matmul (1 psum bank of fp32)
QT = SEQ // NQ                # 2 query tiles per batch
KTT = SEQ // P                # 8 key-token tiles per batch
NV = 384                      # V-projection output tile (2 per 768)
SCALE = HD ** -0.5
CTQ = 2 * D // P              # 12 channel tiles for Q|K


def _emit(tc, xT_d, wqkv_d, wproj_d, bias_d, y_d):
    nc = tc.nc
    from contextlib import ExitStack

    with ExitStack() as ctx:
        consts = ctx.enter_context(tc.tile_pool(name="consts", bufs=1))
        xt_pool = ctx.enter_context(tc.tile_pool(name="xt", bufs=2))
        qk_pool = ctx.enter_context(tc.tile_pool(name="qkT", bufs=2))
        v_pool = ctx.enter_context(tc.tile_pool(name="v", bufs=2))
        ot_pool = ctx.enter_context(tc.tile_pool(name="ot", bufs=2))
        e_pool = ctx.enter_context(tc.tile_pool(name="e", bufs=5))
        dn_pool = ctx.enter_context(tc.tile_pool(name="dn", bufs=2))
        rb_pool = ctx.enter_context(tc.tile_pool(name="rb", bufs=2))
        y_pool = ctx.enter_context(tc.tile_pool(name="y", bufs=2))
        mm_ps = ctx.enter_context(tc.tile_pool(name="mmps", bufs=2, space="PSUM"))
        st_ps = ctx.enter_context(tc.tile_pool(name="stps", bufs=2, space="PSUM"))
        av_ps = ctx.enter_context(tc.tile_pool(name="avps", bufs=2, space="PSUM"))

        # ---- constants ----
        # Per-ktile chunked loads so the first QKV matmul (which only needs
        # the kt=0 slices) starts ~2us in, not after the full 5MB of loads.
        xT_full = xT_d[:].rearrange("(po pi) t -> pi po t", pi=P)  # [128, 6, T]
        wqkv_full = wqkv_d[:].rearrange("(po pi) c -> pi po c", pi=P)

        wqkv_sb = consts.tile([P, KT, 3 * D], BF16)
        xT0_sb = xt_pool.tile([P, KT, SEQ], BF16, tag="xt", name="xT0")
        # tiny first chunks so the very first matmul's operands land fast
        nc.sync.dma_start(out=wqkv_sb[:, 0, 0:P], in_=wqkv_full[:, 0, 0:P])
        nc.sync.dma_start(out=xT0_sb[:, 0, 0:NQ], in_=xT_full[:, 0, 0:NQ])
        nc.sync.dma_start(out=wqkv_sb[:, 0, P : 2 * D], in_=wqkv_full[:, 0, P : 2 * D])
        nc.sync.dma_start(out=xT0_sb[:, 0, NQ:SEQ], in_=xT_full[:, 0, NQ:SEQ])
        for kt in range(1, KT):
            nc.sync.dma_start(out=wqkv_sb[:, kt, : 2 * D], in_=wqkv_full[:, kt, : 2 * D])
            nc.sync.dma_start(out=xT0_sb[:, kt, :], in_=xT_full[:, kt, 0:SEQ])
        for kt in range(KT):
            nc.sync.dma_start(
                out=wqkv_sb[:, kt, 2 * D :], in_=wqkv_full[:, kt, 2 * D :]
            )
        wproj_sb = consts.tile([P, KT, D], BF16)
        nc.sync.dma_start(
            out=wproj_sb, in_=wproj_d[:].rearrange("(po pi) c -> pi po c", pi=P)
        )
        bias_sb = consts.tile([P, D], F32)
        b_ap = bias_d[:]
        bias_bcast = bass.AP(
            tensor=b_ap.tensor, offset=b_ap.offset, ap=[[0, P], *b_ap.ap]
        )
        nc.sync.dma_start(out=bias_sb, in_=bias_bcast)

        # per-batch tiles, allocated lazily
        state = {}

        def batch_state(b):
            if b in state:
                return state[b]
            if b == 0:
                xT_sb = xT0_sb
            else:
                xT_sb = xt_pool.tile([P, KT, SEQ], BF16, tag="xt", name=f"xT{b}")
                nc.sync.dma_start(
                    out=xT_sb, in_=xT_full[:, :, b * SEQ : (b + 1) * SEQ]
                )
            qkT_sb = qk_pool.tile([P, CTQ, SEQ], BF16, tag="qkT", name=f"qkT{b}")
            v_sb = v_pool.tile([P, KTT, H, 2 * HD], BF16, tag="v", name=f"v{b}")
            nc.gpsimd.memset(v_sb[:, :, :, HD : 2 * HD], 1.0)
            outT_sb = ot_pool.tile([P, KT, SEQ], BF16, tag="ot", name=f"ot{b}")
            state[b] = (xT_sb, qkT_sb, v_sb, outT_sb)
            return state[b]

        # ---------- unit generators ----------

        def qkv_units(b):
            """24 Q^T/K^T units + 16 V units; each unit = 6 matmuls + copy."""
            xT_sb, qkT_sb, v_sb, _ = batch_state(b)
            for ct in range(CTQ):
                for qt in range(QT):
                    def u(ct=ct, qt=qt):
                        ps = mm_ps.tile(
                            [P, NQ], F32, tag="mm", name=f"qk{b}_{ct}_{qt}"
                        )
                        for kt in range(KT):
                            nc.tensor.matmul(
                                ps,
                                lhsT=wqkv_sb[:, kt, ct * P : (ct + 1) * P],
                                rhs=xT_sb[:, kt, qt * NQ : (qt + 1) * NQ],
                                start=(kt == 0),
                                stop=(kt == KT - 1),
                            )
                        nc.vector.tensor_copy(
                            out=qkT_sb[:, ct, qt * NQ : (qt + 1) * NQ], in_=ps
                        )
                    yield u
            for tt in range(KTT):
                for nt in range(2):
                    def u(tt=tt, nt=nt):
                        ps = mm_ps.tile(
                            [P, NQ], F32, tag="mm", name=f"v{b}_{tt}_{nt}"
                        )
                        for kt in range(KT):
                            nc.tensor.matmul(
                                ps[:, :NV],
                                lhsT=xT_sb[:, kt, tt * P : (tt + 1) * P],
                                rhs=wqkv_sb[
                                    :, kt, 2 * D + nt * NV : 2 * D + (nt + 1) * NV
                                ],
                                start=(kt == 0),
                                stop=(kt == KT - 1),
                            )
                        nc.vector.tensor_copy(
                            out=v_sb[:, tt, nt * 6 : (nt + 1) * 6, 0:HD],
                            in_=ps[:, :NV].rearrange("p (h d) -> p h d", d=HD),
                        )
                    yield u

        def attn_units(b, fillers=None):
            """12 (head-pair, q-tile) units; ST/AV software-pipelined.
            `fillers`: deque of PE-dense units woven in mid-unit so the PE
            has independent work at the points it would stall on ScalarE."""
            _, qkT_sb, v_sb, outT_sb = batch_state(b)

            def take_filler():
                if fillers:
                    fillers.popleft()()

            for hp in range(H // 2):
                for qt in range(QT):
                    def u(hp=hp, qt=qt):
                        avs = [
                            av_ps.tile(
                                [P, NQ], F32, tag="av", name=f"av{b}_{hp}_{qt}_{i}"
                            )
                            for i in range(2)
                        ]
                        epairs = []

                        def st_exp(kt):
                            # both heads' S^T into one 2-bank psum tile so
                            # the row-tiled pair issues back-to-back, then
                            # one wide exp covers both banks.
                            stp = st_ps.tile(
                                [P, 2, NQ], F32, tag="st",
                                name=f"st{b}_{hp}_{qt}_{kt}",
                            )
                            for hi in range(2):
                                base = hi * HD
                                nc.tensor.matmul(
                                    stp[:, hi, :],
                                    lhsT=qkT_sb[
                                        base : base + HD, KT + hp,
                                        kt * P : (kt + 1) * P,
                                    ],
                                    rhs=qkT_sb[
                                        base : base + HD, hp,
                                        qt * NQ : (qt + 1) * NQ,
                                    ],
                                    start=True,
                                    stop=True,
                                )
                            e_t = e_pool.tile(
                                [P, 2, NQ], BF16, tag="e",
                                name=f"e{b}_{hp}_{qt}_{kt}",
                            )
                            nc.scalar.activation(
                                out=e_t,
                                in_=stp,
                                func=mybir.ActivationFunctionType.Exp,
                                scale=SCALE,
                            )
                            epairs.append(e_t)

                        def av(hi, kt):
                            nc.tensor.matmul(
                                avs[hi],
                                lhsT=v_sb[:, kt, 2 * hp + hi, :],
                                rhs=epairs[kt][:, hi, :],
                                start=(kt == 0),
                                stop=(kt == KTT - 1),
                                skip_group_check=True,
                            )

                        def normalize(hi):
                            base = hi * HD
                            den = dn_pool.tile(
                                [HD, NQ], F32, tag="den",
                                name=f"den{b}_{hp}_{qt}_{hi}",
                            )
                            nc.vector.tensor_copy(
                                out=den, in_=avs[hi][HD : 2 * HD, :]
                            )
                            rb = rb_pool.tile(
                                [HD, NQ], F32, tag="rb",
                                name=f"rb{b}_{hp}_{qt}_{hi}",
                            )
                            nc.vector.reciprocal_approx_fast(out=rb, in_=den)
                            nc.vector.tensor_mul(
                                out=outT_sb[
                                    base : base + HD, hp, qt * NQ : (qt + 1) * NQ
                                ],
                                in0=avs[hi][0:HD, :],
                                in1=rb,
                            )

                        # depth-2 software pipeline: exp(kt) is consumed two
                        # steps later, so the tail AVs never wait on ScalarE.
                        for kt in range(KTT):
                            st_exp(kt)
                            if kt in (3, 6):
                                take_filler()
                            if kt >= 2:
                                av(0, kt - 2)
                                av(1, kt - 2)
                        for kt in (KTT - 2, KTT - 1):
                            av(0, kt)
                            av(1, kt)
                        normalize(0)
                        normalize(1)
                    yield u

        def proj_units(b):
            """8 token-tile units: 12 matmuls + 2 bias adds + 1 DMA out."""
            _, _, _, outT_sb = batch_state(b)
            for tt in range(KTT):
                def u(tt=tt):
                    y_sb = y_pool.tile([P, D], F32, tag="y", name=f"y{b}_{tt}")
                    for n0, nw in ((0, NQ), (NQ, D - NQ)):
                        ps = mm_ps.tile(
                            [P, NQ], F32, tag="mm", name=f"p{b}_{tt}_{n0}"
                        )
                        for dt2 in range(KT):
                            nc.tensor.matmul(
                                ps[:, :nw],
                                lhsT=outT_sb[:, dt2, tt * P : (tt + 1) * P],
                                rhs=wproj_sb[:, dt2, n0 : n0 + nw],
                                start=(dt2 == 0),
                                stop=(dt2 == KT - 1),
                            )
                        nc.vector.tensor_add(
                            out=y_sb[:, n0 : n0 + nw],
                            in0=ps[:, :nw],
                            in1=bias_sb[:, n0 : n0 + nw],
                        )
                    nc.sync.dma_start(
                        out=y_d[b * SEQ + tt * P : b * SEQ + (tt + 1) * P, :],
                        in_=y_sb,
                    )
                yield u

        def interleave(primary, secondary):
            """Emit all primary units, weaving secondary units between them."""
            primary = list(primary)
            secondary = list(secondary)
            n, m = len(primary), len(secondary)
            j = 0
            for i, pu in enumerate(primary):
                pu()
                want = (i + 1) * m // n
                while j < want:
                    secondary[j]()
                    j += 1
            while j < m:
                secondary[j]()
                j += 1

        # ---------- emission schedule ----------
        from collections import deque

        for u in qkv_units(0):
            u()
        # attention(0) is ScalarE-bound: weave batch-1 QKV into the units
        f0 = deque(qkv_units(1))
        for u in attn_units(0, f0):
            u()
            if f0:
                f0.popleft()()
        while f0:
            f0.popleft()()
        # attention(1) likewise hides the batch-0 output projection
        f1 = deque(proj_units(0))
        for i, u in enumerate(attn_units(1, f1)):
            u()
            if f1 and i % 2 == 0:
                f1.popleft()()
        while f1:
            f1.popleft()()
        for u in proj_units(1):
            u()


def _build_program():
    nc = bacc.Bacc()
    xT_d = nc.declare_dram_parameter("xT", [D, T], BF16, isOutput=False)
    wqkv_d = nc.declare_dram_parameter("wqkv", [D, 3 * D], BF16, isOutput=False)
    wproj_d = nc.declare_dram_parameter("wproj", [D, D], BF16, isOutput=False)
    bias_d = nc.declare_dram_parameter("bias", [D], F32, isOutput=False)
    y_d = nc.declare_dram_parameter("y", [T, D], F32, isOutput=True)

    with tile.TileContext(nc) as tc:
        _emit(tc, xT_d, wqkv_d, wproj_d, bias_d, y_d)
    nc.compile()
    return nc


_NC = None


def _get_nc():
    global _NC
    if _NC is None:
        _NC = _build_program()
    return _NC


def _prep_in_maps(x, qkv_w, proj_w, proj_b):
    bf16 = ml_dtypes.bfloat16
    wq = np.ascontiguousarray(np.asarray(qkv_w).astype(bf16))
    wp = np.ascontiguousarray(np.asarray(proj_w).astype(bf16))
    pb = np.ascontiguousarray(np.asarray(proj_b).astype(np.float32))
    x = np.asarray(x)
    in_maps = []
    for c in range(N_CORES):
        xc = x[c * BPC : (c + 1) * BPC].reshape(T, D).astype(bf16)
        xTc = np.ascontiguousarray(xc.T)  # [D, T] bf16
        in_maps.append({"xT": xTc, "wqkv": wq, "wproj": wp, "bias": pb})
    return in_maps


def _run(x, qkv_w, proj_w, proj_b, **spmd_kwargs):
    nc = _get_nc()
    in_maps = _prep_in_maps(x, qkv_w, proj_w, proj_b)
    res = run_bass_kernel_spmd(nc, in_maps, core_ids=list(range(N_CORES)), **spmd_kwargs)
    y = np.stack([res.results[c]["y"] for c in range(N_CORES)])  # [8, T, D]
    return y.reshape(B, SEQ, D).astype(np.float32), res


def kernel(x, qkv_w, proj_w, proj_b):
    y, _ = _run(x, qkv_w, proj_w, proj_b)
    return y


# revision 38
# speedup vs baseline: 1.0239x; 1.0239x over previous
"""Trainium2 Bass kernel: batched multi-head self-attention (B=16, N=1024, D=768, H=12).

Strategy
--------
Data-parallel over the batch: 16 batches / 8 NeuronCores = 2 batches per core.
Each core runs an identical (SPMD) Bass program over its shard.

Per-core math, all matmuls in bf16 with fp32 PSUM accumulation:
  * Host pre-transposes x to xT [D, T] (T = 2048 local tokens) and casts
    x / qkv_w / proj_w to bf16.  Every on-device matmul operand is then in
    its natural layout -- no on-device transposes:
      - Q^T,K^T [c, tok] = matmul(lhsT=qkv_w[:, c-tile], rhs=xT)
      - V [tok, c]       = matmul(lhsT=xT[:, tok-tile], rhs=qkv_w_v)
      - S^T [k, q]       = matmul(lhsT=K^T_h [hd, k-tile], rhs=Q^T_h [hd, q])
        (scores computed TRANSPOSED; no max-subtraction needed: |score|<~6)
      - exp on ScalarE straight out of PSUM, cast to bf16 into SBUF
      - out^T [hd, q]    = matmul(lhsT=[V_h | ones(64)], rhs=expT [k, q]);
        psum rows 64-127 = softmax denominator, replicated for free.
      - normalize via tensor_copy -> reciprocal_approx_fast -> tensor_mul
      - y [tok, e]       = matmul(lhsT=out^T [d, tok-tile], rhs=proj_w) + bias
  * Head pairs share the PE array for S^T: heads 2i/2i+1 live at partition
    bases 0/64 of Q^T/K^T, so their matmuls land on row-tiles (0,0)/(64,0)
    of the 64x128 PE configuration and execute concurrently.
  * The attention stage is ScalarE(exp)-bound, so the emission order
    software-pipelines: within a unit S^T and A@V matmuls interleave, and
    across phases batch-1 QKV / batch-0 proj units are woven between
    attention units to keep the PE busy (and HAM-warm) during exp stalls.

kernel() takes full unsharded inputs, shards on host, runs all 8 cores via
run_bass_kernel_spmd, and re-assembles the full output.
"""

import numpy as np
import ml_dtypes

import concourse.bass as bass
import concourse.mybir as mybir
import concourse.tile as tile
from concourse import bacc
from concourse.bass_utils import run_bass_kernel_spmd

BF16 = mybir.dt.bfloat16
F32 = mybir.dt.float32

N_CORES = 8
B, SEQ, D = 16, 1024, 768
H, HD = 12, 64
BPC = B // N_CORES            # batches per core
T = BPC * SEQ                 # tokens per core
P = 128
KT = D // P                   # 6 contraction sub-tiles of 128
NQ = 512                      # moving free-dim per matmul (1 psum bank of fp32)
QT = SEQ // NQ                # 2 query tiles per batch
KTT = SEQ // P                # 8 key-token tiles per batch
NV = 384                      # V-projection output tile (2 per 768)
SCALE = HD ** -0.5
CTQ = 2 * D // P              # 12 channel tiles for Q|K


def _emit(tc, xT_d, wqkv_d, wproj_d, bias_d, y_d):
    nc = tc.nc
    from contextlib import ExitStack

    with ExitStack() as ctx:
        consts = ctx.enter_context(tc.tile_pool(name="consts", bufs=1))
        xt_pool = ctx.enter_context(tc.tile_pool(name="xt", bufs=2))
        qk_pool = ctx.enter_context(tc.tile_pool(name="qkT", bufs=2))
        v_pool = ctx.enter_context(tc.tile_pool(name="v", bufs=2))
        ot_pool = ctx.enter_context(tc.tile_pool(name="ot", bufs=2))
        e_pool = ctx.enter_context(tc.tile_pool(name="e", bufs=5))
        dn_pool = ctx.enter_context(tc.tile_pool(name="dn", bufs=2))
        rb_pool = ctx.enter_context(tc.tile_pool(name="rb", bufs=2))
        y_pool = ctx.enter_context(tc.tile_pool(name="y", bufs=2))
        mm_ps = ctx.enter_context(tc.tile_pool(name="mmps", bufs=2, space="PSUM"))
        st_ps = ctx.enter_context(tc.tile_pool(name="stps", bufs=2, space="PSUM"))
        av_ps = ctx.enter_context(tc.tile_pool(name="avps", bufs=2, space="PSUM"))

        # ---- constants ----
        # Per-ktile chunked loads so the first QKV matmul (which only needs
        # the kt=0 slices) starts ~2us in, not after the full 5MB of loads.
        xT_full = xT_d[:].rearrange("(po pi) t -> pi po t", pi=P)  # [128, 6, T]
        wqkv_full = wqkv_d[:].rearrange("(po pi) c -> pi po c", pi=P)

        wqkv_sb = consts.tile([P, KT, 3 * D], BF16)
        xT0_sb = xt_pool.tile([P, KT, SEQ], BF16, tag="xt", name="xT0")
        # tiny first chunks so the very first matmul's operands land fast
        nc.sync.dma_start(out=wqkv_sb[:, 0, 0:P], in_=wqkv_full[:, 0, 0:P])
        nc.sync.dma_start(out=xT0_sb[:, 0, 0:NQ], in_=xT_full[:, 0, 0:NQ])
        nc.sync.dma_start(out=wqkv_sb[:, 0, P : 2 * D], in_=wqkv_full[:, 0, P : 2 * D])
        nc.sync.dma_start(out=xT0_sb[:, 0, NQ:SEQ], in_=xT_full[:, 0, NQ:SEQ])
        nc.sync.dma_start(out=wqkv_sb[:, 0, 2 * D :], in_=wqkv_full[:, 0, 2 * D :])
        for kt in range(1, KT):
            nc.sync.dma_start(out=wqkv_sb[:, kt, : 2 * D], in_=wqkv_full[:, kt, : 2 * D])
            nc.sync.dma_start(out=xT0_sb[:, kt, :], in_=xT_full[:, kt, 0:SEQ])
            nc.sync.dma_start(
                out=wqkv_sb[:, kt, 2 * D :], in_=wqkv_full[:, kt, 2 * D :]
            )
        wproj_sb = consts.tile([P, KT, D], BF16)
        nc.sync.dma_start(
            out=wproj_sb, in_=wproj_d[:].rearrange("(po pi) c -> pi po c", pi=P)
        )
        bias_sb = consts.tile([P, D], F32)
        b_ap = bias_d[:]
        bias_bcast = bass.AP(
            tensor=b_ap.tensor, offset=b_ap.offset, ap=[[0, P], *b_ap.ap]
        )
        nc.sync.dma_start(out=bias_sb, in_=bias_bcast)

        # per-batch tiles, allocated lazily
        state = {}

        def batch_state(b):
            if b in state:
                return state[b]
            if b == 0:
                xT_sb = xT0_sb
            else:
                xT_sb = xt_pool.tile([P, KT, SEQ], BF16, tag="xt", name=f"xT{b}")
                nc.sync.dma_start(
                    out=xT_sb, in_=xT_full[:, :, b * SEQ : (b + 1) * SEQ]
                )
            qkT_sb = qk_pool.tile([P, CTQ, SEQ], BF16, tag="qkT", name=f"qkT{b}")
            v_sb = v_pool.tile([P, KTT, H, 2 * HD], BF16, tag="v", name=f"v{b}")
            nc.gpsimd.memset(v_sb[:, :, :, HD : 2 * HD], 1.0)
            outT_sb = ot_pool.tile([P, KT, SEQ], BF16, tag="ot", name=f"ot{b}")
            state[b] = (xT_sb, qkT_sb, v_sb, outT_sb)
            return state[b]

        # ---------- unit generators ----------

        def qkv_units(b):
            """24 Q^T/K^T units + 16 V units; each unit = 6 matmuls + copy."""
            xT_sb, qkT_sb, v_sb, _ = batch_state(b)
            for ct in range(CTQ):
                for qt in range(QT):
                    def u(ct=ct, qt=qt):
                        ps = mm_ps.tile(
                            [P, NQ], F32, tag="mm", name=f"qk{b}_{ct}_{qt}"
                        )
                        for kt in range(KT):
                            nc.tensor.matmul(
                                ps,
                                lhsT=wqkv_sb[:, kt, ct * P : (ct + 1) * P],
                                rhs=xT_sb[:, kt, qt * NQ : (qt + 1) * NQ],
                                start=(kt == 0),
                                stop=(kt == KT - 1),
                            )
                        nc.vector.tensor_copy(
                            out=qkT_sb[:, ct, qt * NQ : (qt + 1) * NQ], in_=ps
                        )
                    yield u
            for tt in range(KTT):
                for nt in range(2):
                    def u(tt=tt, nt=nt):
                        ps = mm_ps.tile(
                            [P, NQ], F32, tag="mm", name=f"v{b}_{tt}_{nt}"
                        )
                        for kt in range(KT):
                            nc.tensor.matmul(
                                ps[:, :NV],
                                lhsT=xT_sb[:, kt, tt * P : (tt + 1) * P],
                                rhs=wqkv_sb[
                                    :, kt, 2 * D + nt * NV : 2 * D + (nt + 1) * NV
                                ],
                                start=(kt == 0),
                                stop=(kt == KT - 1),
                            )
                        nc.vector.tensor_copy(
                            out=v_sb[:, tt, nt * 6 : (nt + 1) * 6, 0:HD],
                            in_=ps[:, :NV].rearrange("p (h d) -> p h d", d=HD),
                        )
                    yield u

        def attn_units(b, fillers=None):
            """12 (head-pair, q-tile) units; ST/AV software-pipelined.
            `fillers`: deque of PE-dense units woven in mid-unit so the PE
            has independent work at the points it would stall on ScalarE."""
            _, qkT_sb, v_sb, outT_sb = batch_state(b)

            def take_filler():
                if fillers:
                    fillers.popleft()()

            for hp in range(H // 2):
                for qt in range(QT):
                    def u(hp=hp, qt=qt):
                        avs = [
                            av_ps.tile(
                                [P, NQ], F32, tag="av", name=f"av{b}_{hp}_{qt}_{i}"
                            )
                            for i in range(2)
                        ]
                        epairs = []

                        def st_exp(kt):
                            # both heads' S^T into one 2-bank psum tile so
                            # the row-tiled pair issues back-to-back, then
                            # one wide exp covers both banks.
                            stp = st_ps.tile(
                                [P, 2, NQ], F32, tag="st",
                                name=f"st{b}_{hp}_{qt}_{kt}",
                            )
                            for hi in range(2):
                                base = hi * HD
                                nc.tensor.matmul(
                                    stp[:, hi, :],
                                    lhsT=qkT_sb[
                                        base : base + HD, KT + hp,
                                        kt * P : (kt + 1) * P,
                                    ],
                                    rhs=qkT_sb[
                                        base : base + HD, hp,
                                        qt * NQ : (qt + 1) * NQ,
                                    ],
                                    start=True,
                                    stop=True,
                                )
                            e_t = e_pool.tile(
                                [P, 2, NQ], BF16, tag="e",
                                name=f"e{b}_{hp}_{qt}_{kt}",
                            )
                            nc.scalar.activation(
                                out=e_t,
                                in_=stp,
                                func=mybir.ActivationFunctionType.Exp,
                                scale=SCALE,
                            )
                            epairs.append(e_t)

                        def av(hi, kt):
                            nc.tensor.matmul(
                                avs[hi],
                                lhsT=v_sb[:, kt, 2 * hp + hi, :],
                                rhs=epairs[kt][:, hi, :],
                                start=(kt == 0),
                                stop=(kt == KTT - 1),
                                skip_group_check=True,
                            )

                        def normalize(hi):
                            base = hi * HD
                            den = dn_pool.tile(
                                [HD, NQ], F32, tag="den",
                                name=f"den{b}_{hp}_{qt}_{hi}",
                            )
                            nc.vector.tensor_copy(
                                out=den, in_=avs[hi][HD : 2 * HD, :]
                            )
                            rb = rb_pool.tile(
                                [HD, NQ], F32, tag="rb",
                                name=f"rb{b}_{hp}_{qt}_{hi}",
                            )
                            nc.vector.reciprocal_approx_fast(out=rb, in_=den)
                            nc.vector.tensor_mul(
                                out=outT_sb[
                                    base : base + HD, hp, qt * NQ : (qt + 1) * NQ
                                ],
                                in0=avs[hi][0:HD, :],
                                in1=rb,
                            )

                        # depth-2 software pipeline: exp(kt) is consumed two
                        # steps later, so the tail AVs never wait on ScalarE.
                        for kt in range(KTT):
                            st_exp(kt)
                            if kt >= 2:
                                av(0, kt - 2)
                                av(1, kt - 2)
                        for kt in (KTT - 2, KTT - 1):
                            av(0, kt)
                            av(1, kt)
                        normalize(0)
                        normalize(1)
                    yield u

        def proj_units(b):
            """8 token-tile units: 12 matmuls + 2 bias adds + 1 DMA out."""
            _, _, _, outT_sb = batch_state(b)
            for tt in range(KTT):
                def u(tt=tt):
                    y_sb = y_pool.tile([P, D], F32, tag="y", name=f"y{b}_{tt}")
                    for n0, nw in ((0, NQ), (NQ, D - NQ)):
                        ps = mm_ps.tile(
                            [P, NQ], F32, tag="mm", name=f"p{b}_{tt}_{n0}"
                        )
                        for dt2 in range(KT):
                            nc.tensor.matmul(
                                ps[:, :nw],
                                lhsT=outT_sb[:, dt2, tt * P : (tt + 1) * P],
                                rhs=wproj_sb[:, dt2, n0 : n0 + nw],
                                start=(dt2 == 0),
                                stop=(dt2 == KT - 1),
                            )
                        nc.vector.tensor_add(
                            out=y_sb[:, n0 : n0 + nw],
                            in0=ps[:, :nw],
                            in1=bias_sb[:, n0 : n0 + nw],
                        )
                    nc.sync.dma_start(
                        out=y_d[b * SEQ + tt * P : b * SEQ + (tt + 1) * P, :],
                        in_=y_sb,
                    )
                yield u

        def interleave(primary, secondary):
            """Emit all primary units, weaving secondary units between them."""
            primary = list(primary)
            secondary = list(secondary)
            n, m = len(primary), len(secondary)
            j = 0
            for i, pu in enumerate(primary):
                pu()
                want = (i + 1) * m // n
                while j < want:
                    secondary[j]()
                    j += 1
            while j < m:
                secondary[j]()
                j += 1

        # ---------- emission schedule ----------
        # Attention for head-pair hp needs qkT channel-tiles {hp, 6+hp} and,
        # for its AVs, the V units of column-half hp//3.  Emit the minimal
        # prefix for hp=0, then start attention immediately and feed the
        # remaining QKV work (batch 0 then batch 1) as PE filler between
        # attention units, keeping ScalarE busy from ~25us onwards.
        u0 = list(qkv_units(0))          # 24 qkT units (ct-major) + 16 V units
        qk0 = u0[:24]                    # qkT unit index = ct*2 + qt
        v0 = u0[24:]                     # V unit index = tt*2 + nt
        v0_first = [v0[tt * 2] for tt in range(KTT)]      # nt=0: heads 0-5
        v0_second = [v0[tt * 2 + 1] for tt in range(KTT)]  # nt=1: heads 6-11

        prefix = qk0[0:2] + qk0[12:14] + v0_first  # ct 0 & 6, V nt=0
        for u in prefix:
            u()
        # remaining batch-0 QKV in the order attention consumes it:
        # Q/K ctiles for hp=1..5 first, V nt=1 before hp=3 needs it.
        rest0 = []
        for hp in range(1, 6):
            rest0 += qk0[2 * hp : 2 * hp + 2] + qk0[12 + 2 * hp : 14 + 2 * hp]
            if hp == 1:
                rest0 += v0_second[:4]
            elif hp == 2:
                rest0 += v0_second[4:]
        fillers = rest0 + list(qkv_units(1))
        interleave(attn_units(0), fillers)
        # attention(1) likewise hides the batch-0 output projection
        interleave(attn_units(1), proj_units(0))
        for u in proj_units(1):
            u()


def _build_program():
    nc = bacc.Bacc()
    xT_d = nc.declare_dram_parameter("xT", [D, T], BF16, isOutput=False)
    wqkv_d = nc.declare_dram_parameter("wqkv", [D, 3 * D], BF16, isOutput=False)
    wproj_d = nc.declare_dram_parameter("wproj", [D, D], BF16, isOutput=False)
    bias_d = nc.declare_dram_parameter("bias", [D], F32, isOutput=False)
    y_d = nc.declare_dram_parameter("y", [T, D], F32, isOutput=True)

    with tile.TileContext(nc) as tc:
        _emit(tc, xT_d, wqkv_d, wproj_d, bias_d, y_d)
    nc.compile()
    return nc


_NC = None


def _get_nc():
    global _NC
    if _NC is None:
        _NC = _build_program()
    return _NC


def _prep_in_maps(x, qkv_w, proj_w, proj_b):
    bf16 = ml_dtypes.bfloat16
    wq = np.ascontiguousarray(np.asarray(qkv_w).astype(bf16))
    wp = np.ascontiguousarray(np.asarray(proj_w).astype(bf16))
    pb = np.ascontiguousarray(np.asarray(proj_b).astype(np.float32))
    x = np.asarray(x)
    in_maps = []
    for c in range(N_CORES):
        xc = x[c * BPC : (c + 1) * BPC].reshape(T, D).astype(bf16)
        xTc = np.ascontiguousarray(xc.T)  # [D, T] bf16
        in_maps.append({"xT": xTc, "wqkv": wq, "wproj": wp, "bias": pb})
    return in_maps


def _run(x, qkv_w, proj_w, proj_b, **spmd_kwargs):
    nc = _get_nc()
    in_maps = _prep_in_maps(x, qkv_w, proj_w, proj_b)
    res = run_bass_kernel_spmd(nc, in_maps, core_ids=list(range(N_CORES)), **spmd_kwargs)
    y = np.stack([res.results[c]["y"] for c in range(N_CORES)])  # [8, T, D]
    return y.reshape(B, SEQ, D).astype(np.float32), res


def kernel(x, qkv_w, proj_w, proj_b):
    y, _ = _run(x, qkv_w, proj_w, proj_b)
    return y


# revision 40
# speedup vs baseline: 1.0333x; 1.0092x over previous
"""Trainium2 Bass kernel: batched multi-head self-attention (B=16, N=1024, D=768, H=12).

Strategy
--------
Data-parallel over the batch: 16 batches / 8 NeuronCores = 2 batches per core.
Each core runs an identical (SPMD) Bass program over its shard.

Per-core math, all matmuls in bf16 with fp32 PSUM accumulation:
  * Host pre-transposes x to xT [D, T] (T = 2048 local tokens) and casts
    x / qkv_w / proj_w to bf16.  Every on-device matmul operand is then in
    its natural layout -- no on-device transposes:
      - Q^T,K^T [c, tok] = matmul(lhsT=qkv_w[:, c-tile], rhs=xT)
      - V [tok, c]       = matmul(lhsT=xT[:, tok-tile], rhs=qkv_w_v)
      - S^T [k, q]       = matmul(lhsT=K^T_h [hd, k-tile], rhs=Q^T_h [hd, q])
        (scores computed TRANSPOSED; no max-subtraction needed: |score|<~6)
      - exp on ScalarE straight out of PSUM, cast to bf16 into SBUF
      - out^T [hd, q]    = matmul(lhsT=[V_h | ones(64)], rhs=expT [k, q]);
        psum rows 64-127 = softmax denominator, replicated for free.
      - normalize via tensor_copy -> reciprocal_approx_fast -> tensor_mul
      - y [tok, e]       = matmul(lhsT=out^T [d, tok-tile], rhs=proj_w) + bias
  * Head pairs share the PE array for S^T: heads 2i/2i+1 live at partition
    bases 0/64 of Q^T/K^T, so their matmuls land on row-tiles (0,0)/(64,0)
    of the 64x128 PE configuration and execute concurrently.
  * The attention stage is ScalarE(exp)-bound, so the emission order
    software-pipelines: within a unit S^T and A@V matmuls interleave, and
    across phases batch-1 QKV / batch-0 proj units are woven between
    attention units to keep the PE busy (and HAM-warm) during exp stalls.

kernel() takes full unsharded inputs, shards on host, runs all 8 cores via
run_bass_kernel_spmd, and re-assembles the full output.
"""

import numpy as np
import ml_dtypes

import concourse.bass as bass
import concourse.mybir as mybir
import concourse.tile as tile
from concourse import bacc
from concourse.bass_utils import run_bass_kernel_spmd

BF16 = mybir.dt.bfloat16
F32 = mybir.dt.float32

N_CORES = 8
B, SEQ, D = 16, 1024, 768
H, HD = 12, 64
BPC = B // N_CORES            # batches per core
T = BPC * SEQ                 # tokens per core
P = 128
KT = D // P                   # 6 contraction sub-tiles of 128
NQ = 512                      # moving free-dim per matmul (1 psum bank of fp32)
QT = SEQ // NQ                # 2 query tiles per batch
KTT = SEQ // P                # 8 key-token tiles per batch
NV = 384                      # V-projection output tile (2 per 768)
SCALE = HD ** -0.5
CTQ = 2 * D // P              # 12 channel tiles for Q|K


def _emit(tc, xT_d, wqkv_d, wproj_d, bias_d, y_d):
    nc = tc.nc
    from contextlib import ExitStack

    with ExitStack() as ctx:
        consts = ctx.enter_context(tc.tile_pool(name="consts", bufs=1))
        xt_pool = ctx.enter_context(tc.tile_pool(name="xt", bufs=2))
        qk_pool = ctx.enter_context(tc.tile_pool(name="qkT", bufs=2))
        v_pool = ctx.enter_context(tc.tile_pool(name="v", bufs=2))
        ot_pool = ctx.enter_context(tc.tile_pool(name="ot", bufs=2))
        e_pool = ctx.enter_context(tc.tile_pool(name="e", bufs=5))
        dn_pool = ctx.enter_context(tc.tile_pool(name="dn", bufs=2))
        rb_pool = ctx.enter_context(tc.tile_pool(name="rb", bufs=2))
        y_pool = ctx.enter_context(tc.tile_pool(name="y", bufs=2))
        mm_ps = ctx.enter_context(tc.tile_pool(name="mmps", bufs=2, space="PSUM"))
        st_ps = ctx.enter_context(tc.tile_pool(name="stps", bufs=2, space="PSUM"))
        av_ps = ctx.enter_context(tc.tile_pool(name="avps", bufs=2, space="PSUM"))

        # ---- constants ----
        # Per-ktile chunked loads so the first QKV matmul (which only needs
        # the kt=0 slices) starts ~2us in, not after the full 5MB of loads.
        xT_full = xT_d[:].rearrange("(po pi) t -> pi po t", pi=P)  # [128, 6, T]
        wqkv_full = wqkv_d[:].rearrange("(po pi) c -> pi po c", pi=P)

        wqkv_sb = consts.tile([P, KT, 3 * D], BF16)
        xT0_sb = xt_pool.tile([P, KT, SEQ], BF16, tag="xt", name="xT0")
        # tiny first chunks so the very first matmul's operands land fast
        nc.sync.dma_start(out=wqkv_sb[:, 0, 0:P], in_=wqkv_full[:, 0, 0:P])
        nc.sync.dma_start(out=xT0_sb[:, 0, 0:NQ], in_=xT_full[:, 0, 0:NQ])
        nc.sync.dma_start(out=wqkv_sb[:, 0, P : 2 * D], in_=wqkv_full[:, 0, P : 2 * D])
        nc.sync.dma_start(out=xT0_sb[:, 0, NQ:SEQ], in_=xT_full[:, 0, NQ:SEQ])
        nc.sync.dma_start(out=wqkv_sb[:, 0, 2 * D :], in_=wqkv_full[:, 0, 2 * D :])
        for kt in range(1, KT):
            nc.sync.dma_start(out=wqkv_sb[:, kt, : 2 * D], in_=wqkv_full[:, kt, : 2 * D])
            nc.sync.dma_start(out=xT0_sb[:, kt, :], in_=xT_full[:, kt, 0:SEQ])
            nc.sync.dma_start(
                out=wqkv_sb[:, kt, 2 * D :], in_=wqkv_full[:, kt, 2 * D :]
            )
        wproj_sb = consts.tile([P, KT, D], BF16)
        nc.sync.dma_start(
            out=wproj_sb, in_=wproj_d[:].rearrange("(po pi) c -> pi po c", pi=P)
        )
        bias_sb = consts.tile([P, D], F32)
        b_ap = bias_d[:]
        bias_bcast = bass.AP(
            tensor=b_ap.tensor, offset=b_ap.offset, ap=[[0, P], *b_ap.ap]
        )
        nc.sync.dma_start(out=bias_sb, in_=bias_bcast)

        # per-batch tiles, allocated lazily
        state = {}

        def batch_state(b):
            if b in state:
                return state[b]
            if b == 0:
                xT_sb = xT0_sb
            else:
                xT_sb = xt_pool.tile([P, KT, SEQ], BF16, tag="xt", name=f"xT{b}")
                nc.sync.dma_start(
                    out=xT_sb, in_=xT_full[:, :, b * SEQ : (b + 1) * SEQ]
                )
            qkT_sb = qk_pool.tile([P, CTQ, SEQ], BF16, tag="qkT", name=f"qkT{b}")
            v_sb = v_pool.tile([P, KTT, H, 2 * HD], BF16, tag="v", name=f"v{b}")
            nc.gpsimd.memset(v_sb[:, :, :, HD : 2 * HD], 1.0)
            outT_sb = ot_pool.tile([P, KT, SEQ], BF16, tag="ot", name=f"ot{b}")
            state[b] = (xT_sb, qkT_sb, v_sb, outT_sb)
            return state[b]

        # ---------- unit generators ----------

        def qkv_units(b):
            """24 Q^T/K^T units + 16 V units; each unit = 6 matmuls + copy."""
            xT_sb, qkT_sb, v_sb, _ = batch_state(b)
            for ct in range(CTQ):
                for qt in range(QT):
                    def u(ct=ct, qt=qt):
                        ps = mm_ps.tile(
                            [P, NQ], F32, tag="mm", name=f"qk{b}_{ct}_{qt}"
                        )
                        for kt in range(KT):
                            nc.tensor.matmul(
                                ps,
                                lhsT=wqkv_sb[:, kt, ct * P : (ct + 1) * P],
                                rhs=xT_sb[:, kt, qt * NQ : (qt + 1) * NQ],
                                start=(kt == 0),
                                stop=(kt == KT - 1),
                            )
                        nc.vector.tensor_copy(
                            out=qkT_sb[:, ct, qt * NQ : (qt + 1) * NQ], in_=ps
                        )
                    yield u
            for tt in range(KTT):
                for nt in range(2):
                    def u(tt=tt, nt=nt):
                        ps = mm_ps.tile(
                            [P, NQ], F32, tag="mm", name=f"v{b}_{tt}_{nt}"
                        )
                        for kt in range(KT):
                            nc.tensor.matmul(
                                ps[:, :NV],
                                lhsT=xT_sb[:, kt, tt * P : (tt + 1) * P],
                                rhs=wqkv_sb[
                                    :, kt, 2 * D + nt * NV : 2 * D + (nt + 1) * NV
                                ],
                                start=(kt == 0),
                                stop=(kt == KT - 1),
                            )
                        nc.vector.tensor_copy(
                            out=v_sb[:, tt, nt * 6 : (nt + 1) * 6, 0:HD],
                            in_=ps[:, :NV].rearrange("p (h d) -> p h d", d=HD),
                        )
                    yield u

        def attn_units(b, fillers=None):
            """12 (head-pair, q-tile) units; ST/AV software-pipelined.
            `fillers`: deque of PE-dense units woven in mid-unit so the PE
            has independent work at the points it would stall on ScalarE."""
            _, qkT_sb, v_sb, outT_sb = batch_state(b)

            def take_filler():
                if fillers:
                    fillers.popleft()()

            for hp in range(H // 2):
                for qt in range(QT):
                    def u(hp=hp, qt=qt):
                        avs = [
                            av_ps.tile(
                                [P, NQ], F32, tag="av", name=f"av{b}_{hp}_{qt}_{i}"
                            )
                            for i in range(2)
                        ]
                        epairs = []

                        def st_exp(kt):
                            # both heads' S^T into one 2-bank psum tile so
                            # the row-tiled pair issues back-to-back, then
                            # one wide exp covers both banks.
                            stp = st_ps.tile(
                                [P, 2, NQ], F32, tag="st",
                                name=f"st{b}_{hp}_{qt}_{kt}",
                            )
                            for hi in range(2):
                                base = hi * HD
                                nc.tensor.matmul(
                                    stp[:, hi, :],
                                    lhsT=qkT_sb[
                                        base : base + HD, KT + hp,
                                        kt * P : (kt + 1) * P,
                                    ],
                                    rhs=qkT_sb[
                                        base : base + HD, hp,
                                        qt * NQ : (qt + 1) * NQ,
                                    ],
                                    start=True,
                                    stop=True,
                                )
                            e_t = e_pool.tile(
                                [P, 2, NQ], BF16, tag="e",
                                name=f"e{b}_{hp}_{qt}_{kt}",
                            )
                            nc.scalar.activation(
                                out=e_t,
                                in_=stp,
                                func=mybir.ActivationFunctionType.Exp,
                                scale=SCALE,
                            )
                            epairs.append(e_t)

                        def av(hi, kt):
                            nc.tensor.matmul(
                                avs[hi],
                                lhsT=v_sb[:, kt, 2 * hp + hi, :],
                                rhs=epairs[kt][:, hi, :],
                                start=(kt == 0),
                                stop=(kt == KTT - 1),
                                skip_group_check=True,
                            )

                        def normalize(hi):
                            base = hi * HD
                            den = dn_pool.tile(
                                [HD, NQ], F32, tag="den",
                                name=f"den{b}_{hp}_{qt}_{hi}",
                            )
                            nc.vector.tensor_copy(
                                out=den, in_=avs[hi][HD : 2 * HD, :]
                            )
                            rb = rb_pool.tile(
                                [HD, NQ], F32, tag="rb",
                                name=f"rb{b}_{hp}_{qt}_{hi}",
                            )
                            nc.vector.reciprocal_approx_fast(out=rb, in_=den)
                            nc.vector.tensor_mul(
                                out=outT_sb[
                                    base : base + HD, hp, qt * NQ : (qt + 1) * NQ
                                ],
                                in0=avs[hi][0:HD, :],
                                in1=rb,
                            )

                        # depth-2 software pipeline: exp(kt) is consumed two
                        # steps later, so the tail AVs never wait on ScalarE.
                        for kt in range(KTT):
                            st_exp(kt)
                            if kt == 1:
                                # one PE-dense filler exactly where the unit
                                # would stall waiting for its first exp to
                                # free an S^T psum slot
                                take_filler()
                            if kt >= 2:
                                av(0, kt - 2)
                                av(1, kt - 2)
                        for kt in (KTT - 2, KTT - 1):
                            av(0, kt)
                            av(1, kt)
                        normalize(0)
                        normalize(1)
                    yield u

        def proj_units(b):
            """8 token-tile units: 12 matmuls + 2 bias adds + 1 DMA out."""
            _, _, _, outT_sb = batch_state(b)
            for tt in range(KTT):
                def u(tt=tt):
                    y_sb = y_pool.tile([P, D], F32, tag="y", name=f"y{b}_{tt}")
                    for n0, nw in ((0, NQ), (NQ, D - NQ)):
                        ps = mm_ps.tile(
                            [P, NQ], F32, tag="mm", name=f"p{b}_{tt}_{n0}"
                        )
                        for dt2 in range(KT):
                            nc.tensor.matmul(
                                ps[:, :nw],
                                lhsT=outT_sb[:, dt2, tt * P : (tt + 1) * P],
                                rhs=wproj_sb[:, dt2, n0 : n0 + nw],
                                start=(dt2 == 0),
                                stop=(dt2 == KT - 1),
                            )
                        nc.vector.tensor_add(
                            out=y_sb[:, n0 : n0 + nw],
                            in0=ps[:, :nw],
                            in1=bias_sb[:, n0 : n0 + nw],
                        )
                    nc.sync.dma_start(
                        out=y_d[b * SEQ + tt * P : b * SEQ + (tt + 1) * P, :],
                        in_=y_sb,
                    )
                yield u

        def interleave(primary, secondary):
            """Emit all primary units, weaving secondary units between them."""
            primary = list(primary)
            secondary = list(secondary)
            n, m = len(primary), len(secondary)
            j = 0
            for i, pu in enumerate(primary):
                pu()
                want = (i + 1) * m // n
                while j < want:
                    secondary[j]()
                    j += 1
            while j < m:
                secondary[j]()
                j += 1

        # ---------- emission schedule ----------
        from collections import deque

        for u in qkv_units(0):
            u()
        # attention(0) is ScalarE-bound: fill PE slack with batch-1 QKV
        # (one filler inside each unit at the first-exp stall, the rest
        # woven between units)
        f0 = deque(qkv_units(1))
        for u in attn_units(0, f0):
            u()
            for _ in range(2):
                if f0:
                    f0.popleft()()
        while f0:
            f0.popleft()()
        # attention(1) likewise hides the batch-0 output projection
        f1 = deque(proj_units(0))
        for u in attn_units(1, f1):
            u()
        while f1:
            f1.popleft()()
        for u in proj_units(1):
            u()


def _build_program():
    nc = bacc.Bacc()
    xT_d = nc.declare_dram_parameter("xT", [D, T], BF16, isOutput=False)
    wqkv_d = nc.declare_dram_parameter("wqkv", [D, 3 * D], BF16, isOutput=False)
    wproj_d = nc.declare_dram_parameter("wproj", [D, D], BF16, isOutput=False)
    bias_d = nc.declare_dram_parameter("bias", [D], F32, isOutput=False)
    y_d = nc.declare_dram_parameter("y", [T, D], F32, isOutput=True)

    with tile.TileContext(nc) as tc:
        _emit(tc, xT_d, wqkv_d, wproj_d, bias_d, y_d)
    nc.compile()
    return nc


_NC = None


def _get_nc():
    global _NC
    if _NC is None:
        _NC = _build_program()
    return _NC


def _prep_in_maps(x, qkv_w, proj_w, proj_b):
    bf16 = ml_dtypes.bfloat16
    wq = np.ascontiguousarray(np.asarray(qkv_w).astype(bf16))
    wp = np.ascontiguousarray(np.asarray(proj_w).astype(bf16))
    pb = np.ascontiguousarray(np.asarray(proj_b).astype(np.float32))
    x = np.asarray(x)
    in_maps = []
    for c in range(N_CORES):
        xc = x[c * BPC : (c + 1) * BPC].reshape(T, D).astype(bf16)
        xTc = np.ascontiguousarray(xc.T)  # [D, T] bf16
        in_maps.append({"xT": xTc, "wqkv": wq, "wproj": wp, "bias": pb})
    return in_maps


def _run(x, qkv_w, proj_w, proj_b, **spmd_kwargs):
    nc = _get_nc()
    in_maps = _prep_in_maps(x, qkv_w, proj_w, proj_b)
    res = run_bass_kernel_spmd(nc, in_maps, core_ids=list(range(N_CORES)), **spmd_kwargs)
    y = np.stack([res.results[c]["y"] for c in range(N_CORES)])  # [8, T, D]
    return y.reshape(B, SEQ, D).astype(np.float32), res


def kernel(x, qkv_w, proj_w, proj_b):
    y, _ = _run(x, qkv_w, proj_w, proj_b)
    return y
